# revision 1
# baseline (speedup 1.0000x reference)
"""Boltzformer decoder mask kernel for Trainium2 (8 NeuronCores, SPMD).

Full-input contract: kernel(**inputs) takes the unsharded tensors from
setup_inputs() and returns the full [16, 1024, 1024] float32 output.

Sharding: data-parallel over the B*H=16 leading dim. Core c handles batch
c//4 and the two head-slices (2c, 2c+1). The attention map is identical
across heads within a batch, so each core computes LN -> MLP -> me@me^T ->
smask chain once, and only the rand-dependent tail twice.

The reference output is fp32-quantized: out = (1 - sigmoid(100(attn-0.5)))
* boltz, and for this regime every value is a multiple of 2^-23 (jax's
1/(1+e^-z) rounds (1+e^-z) on the 2^-23 grid) with only a handful nonzero.
The fast16 variant exploits this:

- fp16 matmuls (4x PE rate vs fp32): score error ~6e-5, two orders below
  the ~2e-3 bucket-flip threshold (verified on the actual inputs, host
  guard re-checks).
- smask tail via tanh+exp only (one ACT table set, no sigmoid<->exp table
  reloads): t/2 = exp(-50*tanh(s/2) - ln2), written directly to fp16 whose
  SUBNORMAL grid (2^-24) reproduces the reference's 2^-23 bucket grid for
  t/2 exactly (t < 2^-14 guaranteed by the guard).
- boltz = (1+tanh(50(1-r)))/2: the +1 runs early (Pool for head 0, DVE
  for late head-1 chunks); the final multiply (u+1)*tq is a 2x-mode DVE
  tensor_tensor written bf16, with the three earliest head-1 chunks on
  Pool to keep the DVE stream from backlogging past the ACT stream.
- only the upper-right block triangle of the symmetric map is computed
  directly (chunk 7 first, fully; then chunks 0..6 with direct columns
  [qc*128, 896)); the remaining blocks are PE-transposed copies of the
  already-quantized tq, which is bit-exact. Scores matmuls are split at
  PSUM bank boundaries (512 f32) -- unaligned matmul outputs silently
  corrupt on hardware.
- x and rand ship as fp16, w as fp16, out ships as bf16 (pure relative
  error ~2^-9): 8.9MB of DMA per core instead of 18.5MB.
- all ACT work (sqrt early, then tanh+exp only) needs just two activation
  table loads; the baseline's sigmoid<->exp interleaving reloaded tables
  ~16 times (~1.3us each on hardware, invisible to the cost-model sim).
- engine legality on TRN2: GPSIMD/Pool cannot touch PSUM, so every PSUM
  drain (transpose copies, MLP bias+relu, mirror copies) runs on DVE;
  Pool gets the SBUF-only work (xn, +1 adds).
"""

import math

import numpy as np

B = 2
Q = 1024
D = 256
NUM_HEADS = 8
N_CORES = 8
HEADS_PER_CORE = 2
THRESHOLD = 0.5
N_SAMPLES = int(Q * 0.1)  # 102
LN_EPS = 1e-4
BP_EPS = 1e-6
P = 128  # SBUF partitions
QC = Q // P  # 8 row-chunks per map
FMAP = QC * Q  # [1024,1024] map stored as [128, 8192]
LN2F = float(np.log(np.float32(2.0)))

_BUILD_CACHE = {}
_LEGALIZE = True


def _legalize_waits(nc):
    """TRN2 instruction structs carry only ONE inline sync-wait slot (fp32
    self-loading matmuls, activations, DVE tensor ops, DMA descriptors
    alike). Tile attaches multi-waits; legalize by hoisting the excess waits
    onto standalone same-engine NoOps right before the instruction (the
    raw-bass "wait_ge then op" idiom). Walrus partitions blocks by engine
    preserving order, so a NoOp inserted directly before stays ahead in that
    engine's queue -- semantics are preserved exactly."""
    import concourse.mybir as mybir
    import bass_rust

    skip = ("InstDmaTransposeAnt", "InstTriggerDma")
    for blk in nc.m.functions[0].blocks:
        out_list = []
        for ins in blk.instructions:
            si = getattr(ins, "sync_info", None)
            eng = getattr(ins, "engine", None)
            if (
                si is not None
                and eng is not None
                and type(ins).__name__ not in skip
                and len(si.on_wait) > 1
            ):
                waits = list(si.on_wait)
                for j, w in enumerate(waits[:-1]):
                    nop = mybir.InstNoOp(name=f"{ins.name}-ws{j}", ins=[], outs=[])
                    nop.engine = eng
                    nop.sync_info = bass_rust.SyncInfo(on_wait=[w], on_update=[])
                    out_list.append(nop)
                si.on_wait = [waits[-1]]
            out_list.append(ins)
        blk.instructions = out_list
    return nc


def _build_fast16():
    """Fast variant: fp16 matmuls, tanh/exp-only ACT work, fp16 rand in,
    bf16 out. Valid when the host guard confirms the bp==0 regime with
    margin (all scores > 0.45)."""
    import contextlib

    import concourse.bass as bass
    import concourse.tile as tile
    import concourse.mybir as mybir

    fp32 = mybir.dt.float32
    fp16 = mybir.dt.float16
    bf16 = mybir.dt.bfloat16
    AF = mybir.ActivationFunctionType
    OP = mybir.AluOpType

    nc = bass.Bass("TRN2", target_bir_lowering=False)

    x_d = nc.dram_tensor("x", [Q, D], fp16, kind="ExternalInput")
    w_d = nc.dram_tensor("w", [3, D, D], fp16, kind="ExternalInput")
    b_d = nc.dram_tensor("b", [3, D], fp32, kind="ExternalInput")
    rand_d = nc.dram_tensor("rand", [HEADS_PER_CORE, Q, Q], fp16, kind="ExternalInput")
    out_d = nc.dram_tensor("out", [HEADS_PER_CORE, Q, Q], bf16, kind="ExternalOutput")

    with tile.TileContext(nc) as tc:
        ctx = contextlib.ExitStack()
        with ctx:
            consts = ctx.enter_context(tc.tile_pool(name="consts", bufs=1))
            smalls = ctx.enter_context(tc.tile_pool(name="smalls", bufs=1))
            acts = ctx.enter_context(tc.tile_pool(name="acts", bufs=4))
            maps = ctx.enter_context(tc.tile_pool(name="maps", bufs=5))

            # ---- input DMAs (emission order == DMA priority order) ----
            x_sb = smalls.tile([P, QC, D], fp16)
            x_r = x_d[:, :].rearrange("(t p) d -> p t d", p=P)
            rand_sb = [
                maps.tile([P, FMAP], fp16, tag="maps", name=f"rand_sb{h}")
                for h in range(2)
            ]
            rand_r = rand_d[:, :, :].rearrange("h (t p) k -> h p t k", p=P)

            def rand_dma(h, t):
                nc.sync.dma_start(
                    out=rand_sb[h].rearrange("p (t k) -> p t k", k=Q)[:, t, :],
                    in_=rand_r[h, :, t, :],
                )

            # x in 2-chunk pieces (SP issue cadence is 650ns; 364ns chunks
            # would be issue-bound), then first rand chunks, then weights.
            for t0 in range(0, QC, 2):
                nc.sync.dma_start(
                    out=x_sb[:, t0 : t0 + 2, :], in_=x_r[:, t0 : t0 + 2, :]
                )
            rand_dma(0, 0)
            rand_dma(0, 1)
            w_sb = consts.tile([P, 3, 2, D], fp16)
            nc.sync.dma_start(
                out=w_sb, in_=w_d[:, :, :].rearrange("l (kc p) f -> p l kc f", p=P)
            )
            b_sb = consts.tile([P, 3, 2], fp32)
            nc.sync.dma_start(
                out=b_sb, in_=b_d[:, :].rearrange("l (c p) -> p l c", p=P)
            )
            for t in range(2, QC):
                rand_dma(0, t)
            # h1 chunks 7,5,6 first: their boltz runs in the pre-scores ACT
            # window (chunks processed in order 7,0..6, so 7/5/6 mults come
            # at the stream's start/end)
            for t in (7, 5, 6, 0, 1, 2, 3, 4):
                rand_dma(1, t)

            identity = consts.tile([P, P], fp16)
            nc.gpsimd.memset(identity, 0.0)
            nc.gpsimd.affine_select(
                out=identity,
                in_=identity,
                compare_op=OP.not_equal,
                fill=1.0,
                base=0,
                pattern=[[-1, P]],
                channel_multiplier=1,
            )

            # PE pstate warmup: the Tensor engine needs ~3us of continuous
            # work to reach max clock. Burn the input-DMA wait on dummy
            # self-transposes so the MLP/scores run at full speed.
            with tc.tile_pool(name="wup", bufs=2, space="PSUM") as wup:
                wtile = [
                    wup.tile([P, P], fp16, tag="wu", name=f"wu{i}")
                    for i in range(2)
                ]
                for i in range(26):
                    nc.tensor.transpose(wtile[i % 2], identity, identity)

            # ---- Phase A: LayerNorm (row-major, per 128-row tile) ----
            stats = smalls.tile([P, QC, 6], fp32)
            mv = smalls.tile([P, QC, 2], fp32)
            sd = smalls.tile([P, QC], fp32)
            rstd = smalls.tile([P, QC], fp32)
            eps_t = smalls.tile([P, 1], fp32)
            nc.vector.memset(eps_t, LN_EPS)
            c50_t = smalls.tile([P, 1], fp32)
            nc.vector.memset(c50_t, 50.0)
            cml2_t = smalls.tile([P, 1], fp32)
            nc.vector.memset(cml2_t, -LN2F)
            one_t = smalls.tile([P, 1], fp32)
            nc.vector.memset(one_t, 1.0)
            xn = [
                acts.tile([P, QC // 2, D], fp16, tag="actT", name=f"xn{i}")
                for i in range(2)
            ]
            # stats chase the x DMA; sqrt/recip/xn pipelined in two halves
            # so the first transposes start before the last x chunk lands
            H = QC // 2
            for t in range(H):
                nc.vector.bn_stats(out=stats[:, t, :], in_=x_sb[:, t, :])
            for t in range(H):
                nc.vector.bn_aggr(out=mv[:, t, :], in_=stats[:, t, :])
            nc.scalar.activation(
                out=sd[:, :H], in_=mv[:, :H, 1], func=AF.Sqrt,
                bias=eps_t, scale=1.0,
            )
            for t in range(H, QC):
                nc.vector.bn_stats(out=stats[:, t, :], in_=x_sb[:, t, :])
            for t in range(H, QC):
                nc.vector.bn_aggr(out=mv[:, t, :], in_=stats[:, t, :])
            nc.scalar.activation(
                out=sd[:, H:], in_=mv[:, H:, 1], func=AF.Sqrt,
                bias=eps_t, scale=1.0,
            )
            nc.vector.reciprocal(out=rstd[:, :H], in_=sd[:, :H])
            for t in range(H):
                nc.vector.tensor_scalar(
                    out=xn[t // 4][:, t % 4, :],
                    in0=x_sb[:, t, :],
                    scalar1=mv[:, t, 0:1],
                    scalar2=rstd[:, t : t + 1],
                    op0=OP.subtract,
                    op1=OP.mult,
                )
            nc.vector.reciprocal(out=rstd[:, H:], in_=sd[:, H:])
            for t in range(H, QC):
                nc.vector.tensor_scalar(
                    out=xn[t // 4][:, t % 4, :],
                    in0=x_sb[:, t, :],
                    scalar1=mv[:, t, 0:1],
                    scalar2=rstd[:, t : t + 1],
                    op0=OP.subtract,
                    op1=OP.mult,
                )

            # absorb the bias-DMA tick on DVE so MLP bias ops carry <=1 wait
            b_abs = smalls.tile([P, 1], fp32)
            nc.vector.tensor_copy(out=b_abs, in_=b_sb[:, 0, 0:1])

            # ---- Phase B: transpose xn -> xT (feature-major, fp16) ----
            # 4 transposes share one [P,512] PSUM tile; a single 2x-mode
            # 512-wide copy drains each group.
            xT = [acts.tile([P, Q], fp16, tag="actT", name=f"xT{h}") for h in range(2)]
            with tc.tile_pool(name="tpsum", bufs=2, space="PSUM") as tpsum:
                # late PE warmup gated on xn: keeps the PE clock ramping
                # continuously into the real transposes and the MLP
                with tc.tile_pool(name="wup2", bufs=2, space="PSUM") as wup2:
                    w2t = [
                        wup2.tile([P, P], fp16, tag="wu2", name=f"wu2_{i}")
                        for i in range(2)
                    ]
                    for i in range(22):
                        nc.tensor.transpose(
                            w2t[i % 2], xn[0][:, 0, 0:P], identity
                        )
                for g in range(4):
                    h, t0 = g % 2, (g // 2) * 4
                    pst = tpsum.tile([P, 4 * P], fp16, tag="tp", name=f"pst{g}")
                    for i in range(4):
                        t = t0 + i
                        nc.tensor.transpose(
                            pst[:, i * P : (i + 1) * P],
                            xn[t // 4][:, t % 4, h * P : (h + 1) * P],
                            identity,
                        )
                    nc.vector.tensor_copy(
                        out=xT[h][:, t0 * P : (t0 + 4) * P], in_=pst
                    )

            # ---- Phase C: 3-layer MLP in feature-major fp16 ----
            # [P,512] drains ordered rc-first so the next layer's matmuls
            # start after two drains instead of the full layer barrier.
            with tc.tile_pool(name="mlpp", bufs=4, space="PSUM") as mlpp:
                cur = xT
                for layer in range(3):
                    nxt = [
                        acts.tile([P, Q], fp16, tag="actT", name=f"y{layer}T{f2}")
                        for f2 in range(2)
                    ]
                    pss = {}
                    for fc in range(2):
                        for rc in range(2):
                            ps = mlpp.tile(
                                [P, 512], fp32, tag="mm", name=f"mlp{layer}_{fc}{rc}"
                            )
                            for kc in range(2):
                                nc.tensor.matmul(
                                    ps,
                                    lhsT=w_sb[:, layer, kc, fc * P : (fc + 1) * P],
                                    rhs=cur[kc][:, rc * 512 : (rc + 1) * 512],
                                    start=(kc == 0),
                                    stop=(kc == 1),
                                )
                            pss[(fc, rc)] = ps
                    for rc in range(2):
                        for fc in range(2):
                            if layer < 2:
                                nc.vector.tensor_scalar(
                                    out=nxt[fc][:, rc * 512 : (rc + 1) * 512],
                                    in0=pss[(fc, rc)],
                                    scalar1=b_sb[:, layer, fc : fc + 1],
                                    scalar2=0.0,
                                    op0=OP.add,
                                    op1=OP.max,
                                )
                            else:
                                nc.vector.tensor_scalar(
                                    out=nxt[fc][:, rc * 512 : (rc + 1) * 512],
                                    in0=pss[(fc, rc)],
                                    scalar1=b_sb[:, layer, fc : fc + 1],
                                    scalar2=None,
                                    op0=OP.add,
                                )
                    cur = nxt
            meT = cur  # [2][128, 1024] feature-major me^T, fp16

            # ---- Phase D: scores -> tq (tanh+exp, one ACT table) ----
            # tq = fp16(exp(-50*tanh(s/2) - ln2)) == reference bucket of
            # (1-sigmoid(100(attn-0.5)))/2 on the fp16 subnormal grid.
            # The map is symmetric (jax's einsum makes s[q,k] bitwise equal
            # to s[k,q]), so only columns >= qc*128 are computed directly;
            # the strict-lower blocks are PE-transposed copies of the
            # quantized tq -- bit-exact mirroring.
            # boltz: u = tanh(50(1-r)) in place over rand (fp16).
            # out = (u+1)*tq per head (DVE / Pool), written bf16.
            tq = maps.tile([P, FMAP], fp16, tag="maps", name="tq")
            outm = [
                maps.tile([P, FMAP], bf16, tag="maps", name=f"outm{h}")
                for h in range(2)
            ]
            ut = [
                acts.tile([P, Q], fp32, tag="attn", name=f"ut{i}")
                for i in range(2)
            ]
            out_r = out_d[:, :, :].rearrange("h (t p) k -> h p t k", p=P)
            spsum = ctx.enter_context(
                tc.tile_pool(name="spsum", bufs=3, space="PSUM")
            )
            mirp = ctx.enter_context(
                tc.tile_pool(name="mirp", bufs=2, space="PSUM")
            )

            def boltz(h, qc, n=1):
                sl = slice(qc * Q, (qc + n) * Q)
                nc.scalar.activation(
                    out=rand_sb[h][:, sl],
                    in_=rand_sb[h][:, sl],
                    func=AF.Tanh,
                    scale=-50.0,
                    bias=c50_t,
                )

            # chunk 7 is processed FIRST with all 1024 columns direct; then
            # chunks 0..6 compute direct cols [qc*128, 896) and mirror the
            # left blocks from earlier chunks plus the last block from
            # chunk 7. The ACT stream narrows toward the end, so the tail
            # after the final (128-wide) exp is minimal.
            def drange(qc):
                return (0, Q) if qc == QC - 1 else (qc * P, Q - P)

            def scores(qc):
                c0, c1 = drange(qc)
                ps = spsum.tile([P, Q], fp32, tag="sp", name=f"sps{qc}")
                # matmul outputs must not cross a PSUM bank (512 f32)
                bounds = sorted({c0, c1} | {b for b in (512,) if c0 < b < c1})
                for n0, n1 in zip(bounds[:-1], bounds[1:]):
                    for kc in range(2):
                        nc.tensor.matmul(
                            ps[:, n0:n1],
                            lhsT=meT[kc][:, qc * P : (qc + 1) * P],
                            rhs=meT[kc][:, n0:n1],
                            start=(kc == 0),
                            stop=(kc == 1),
                        )
                return ps

            def attn_tanh(qc, ps):
                c0, c1 = drange(qc)
                nc.scalar.activation(
                    out=ut[qc % 2][:, c0:c1],
                    in_=ps[:, c0:c1],
                    func=AF.Tanh,
                    scale=1.0 / 32.0,
                )

            def exp_tq(qc):
                c0, c1 = drange(qc)
                nc.scalar.activation(
                    out=tq[:, qc * Q + c0 : qc * Q + c1],
                    in_=ut[qc % 2][:, c0:c1],
                    func=AF.Exp,
                    scale=-50.0,
                    bias=cml2_t,
                )

            def add1(h, qc, n=1, eng=None):
                # u1 = u + 1 in place, right after boltz lands; on Pool
                # (SBUF-only op) so the DVE keeps its PSUM/mult budget
                sl = slice(qc * Q, (qc + n) * Q)
                if eng is None:
                    eng = nc.gpsimd
                eng.tensor_scalar(
                    out=rand_sb[h][:, sl],
                    in0=rand_sb[h][:, sl],
                    scalar1=1.0,
                    scalar2=None,
                    op0=OP.add,
                )

            def mirror_from(j):
                # after exp_tq(j): transpose each direct block (j -> i) into
                # the mirrored position of chunk i. j == 7 feeds every other
                # chunk's last block; j < 7 feeds later chunks' left blocks.
                if j == QC - 1:
                    dsts = [(i, QC - 1) for i in range(QC - 1)]
                else:
                    dsts = [(i, j) for i in range(j + 1, QC - 1)]
                for i, col in dsts:
                    mp_ = mirp.tile([P, P], fp16, tag="mp", name=f"mir{j}_{i}")
                    nc.tensor.transpose(
                        mp_,
                        tq[:, j * Q + i * P : j * Q + (i + 1) * P],
                        identity,
                    )
                    nc.vector.tensor_copy(
                        out=tq[:, i * Q + col * P : i * Q + (col + 1) * P],
                        in_=mp_,
                    )

            def mult_dma(h, qc, n=1, eng=None):
                # out = u1 * tq (DVE tensor_tensor 2x mode; a few early
                # head-1 chunks go to Pool to trim the DVE stream backlog)
                sl = slice(qc * Q, (qc + n) * Q)
                if eng is None:
                    eng = nc.vector
                eng.tensor_tensor(
                    out=outm[h][:, sl],
                    in0=rand_sb[h][:, sl],
                    in1=tq[:, sl],
                    op=OP.mult,
                )
                nc.sync.dma_start(
                    out=out_r[h, :, qc : qc + n, :], in_=outm[h][:, sl]
                )

            # ACT queue (single exp+tanh table after the early sqrt):
            # boltz h0 fills the rand-DMA window, then per-chunk tanh/exp
            # as scores land, with boltz h1 slotted in so head-1 out DMAs
            # stream too.
            boltz(0, 0)
            boltz(0, 1)
            boltz(0, 2)
            boltz(0, 3)
            boltz(0, 4, n=2)
            boltz(0, 6, n=2)
            # fill the pre-scores ACT window with the head-1 boltz chunks
            # whose mults sit at the stream's start (7) and end (5, 6)
            early_h1 = (7, 5, 6)
            boltz(1, 7)
            boltz(1, 5, n=2)
            # +1 adds: head 0 on DVE pairs (4x immediate, fits the DVE
            # front-end idle before the mult stream); head-1 early on Pool
            for c in (0, 2, 4, 6):
                add1(0, c, n=2, eng=nc.vector)
            add1(1, 7)
            add1(1, 5)
            add1(1, 6)

            # software-pipelined emission: scores are issued two rounds
            # ahead of the mirror transposes so the in-order PE queue never
            # head-blocks upcoming chunks' matmuls behind mirror transposes
            # that wait on the ACT stream (spsum ring is 3 deep to match).
            PROC = (7, 0, 1, 2, 3, 4, 5, 6)
            psd = {PROC[0]: scores(PROC[0]), PROC[1]: scores(PROC[1])}
            for idx, qc in enumerate(PROC):
                attn_tanh(qc, psd.pop(qc))
                exp_tq(qc)
                if idx + 2 < len(PROC):
                    psd[PROC[idx + 2]] = scores(PROC[idx + 2])
                mirror_from(qc)
                mult_dma(0, qc)
                if qc in (0, 2):
                    boltz(1, qc, n=2)
                    add1(1, qc, n=2, eng=nc.vector)
                # early head-1 mults on Pool to trim the DVE stream backlog
                if qc != 4:
                    mult_dma(1, qc, eng=nc.gpsimd if qc in (7, 0, 1, 2) else None)

            # chunk 4's head-1 boltz/mult moved past the last exps: its
            # 1038ns tanh no longer delays the final four tanh/exp ops
            boltz(1, 4)
            add1(1, 4, eng=nc.vector)
            mult_dma(1, 4)

    return _legalize_waits(nc) if _LEGALIZE else nc


def _build_full(layer_id: int):
    """General path: full Boltzmann chain in f32 (baseline implementation)."""
    import contextlib

    import concourse.bass as bass
    import concourse.tile as tile
    import concourse.mybir as mybir

    fp32 = mybir.dt.float32
    AF = mybir.ActivationFunctionType
    OP = mybir.AluOpType

    exp_scale = 2.0 + float(layer_id)  # attn / temp == attn * (2 + layer_id)

    nc = bass.Bass("TRN2", target_bir_lowering=False)

    x_d = nc.dram_tensor("x", [Q, D], fp32, kind="ExternalInput")
    w_d = nc.dram_tensor("w", [3, D, D], fp32, kind="ExternalInput")
    b_d = nc.dram_tensor("b", [3, D], fp32, kind="ExternalInput")
    rand_d = nc.dram_tensor("rand", [HEADS_PER_CORE, Q, Q], fp32, kind="ExternalInput")
    out_d = nc.dram_tensor("out", [HEADS_PER_CORE, Q, Q], fp32, kind="ExternalOutput")

    with tile.TileContext(nc) as tc:
        ctx = contextlib.ExitStack()
        with ctx:
            consts = ctx.enter_context(tc.tile_pool(name="consts", bufs=1))
            smalls = ctx.enter_context(tc.tile_pool(name="smalls", bufs=1))
            acts = ctx.enter_context(tc.tile_pool(name="acts", bufs=4))
            maps = ctx.enter_context(tc.tile_pool(name="maps", bufs=5))

            x_sb = smalls.tile([P, QC, D], fp32)
            x_r = x_d[:, :].rearrange("(t p) d -> p t d", p=P)
            for t in range(QC):
                nc.sync.dma_start(out=x_sb[:, t, :], in_=x_r[:, t, :])
            w_sb = consts.tile([P, 3, 2, D], fp32)
            nc.sync.dma_start(
                out=w_sb, in_=w_d[:, :, :].rearrange("l (kc p) f -> p l kc f", p=P)
            )
            b_sb = consts.tile([P, 3, 2], fp32)
            nc.sync.dma_start(
                out=b_sb, in_=b_d[:, :].rearrange("l (c p) -> p l c", p=P)
            )
            rand_sb = [
                maps.tile([P, FMAP], fp32, tag="maps", name=f"rand_sb{h}")
                for h in range(2)
            ]
            for h in range(2):
                nc.sync.dma_start(
                    out=rand_sb[h].rearrange("p (t k) -> p t k", k=Q),
                    in_=rand_d[h, :, :].rearrange("(t p) k -> p t k", p=P),
                )

            identity = consts.tile([P, P], fp32)
            nc.gpsimd.memset(identity, 0.0)
            nc.gpsimd.affine_select(
                out=identity,
                in_=identity,
                compare_op=OP.not_equal,
                fill=1.0,
                base=0,
                pattern=[[-1, P]],
                channel_multiplier=1,
            )

            stats = smalls.tile([P, QC, 6], fp32)
            mv = smalls.tile([P, QC, 2], fp32)
            sd = smalls.tile([P, QC], fp32)
            rstd = smalls.tile([P, QC], fp32)
            eps_t = smalls.tile([P, 1], fp32)
            nc.vector.memset(eps_t, LN_EPS)
            c50_t = smalls.tile([P, 1], fp32)
            nc.vector.memset(c50_t, 50.0)
            c100_t = smalls.tile([P, 1], fp32)
            nc.vector.memset(c100_t, 100.0)
            xn = [
                acts.tile([P, QC // 2, D], fp32, tag="actT", name=f"xn{i}")
                for i in range(2)
            ]
            for t in range(QC):
                nc.vector.bn_stats(out=stats[:, t, :], in_=x_sb[:, t, :])
                nc.vector.bn_aggr(out=mv[:, t, :], in_=stats[:, t, :])
                nc.scalar.activation(
                    out=sd[:, t : t + 1],
                    in_=mv[:, t, 1:2],
                    func=AF.Sqrt,
                    bias=eps_t,
                    scale=1.0,
                )
                nc.vector.reciprocal(
                    out=rstd[:, t : t + 1], in_=sd[:, t : t + 1]
                )
                nc.vector.tensor_scalar(
                    out=xn[t // 4][:, t % 4, :],
                    in0=x_sb[:, t, :],
                    scalar1=mv[:, t, 0:1],
                    scalar2=rstd[:, t : t + 1],
                    op0=OP.subtract,
                    op1=OP.mult,
                )

            b_abs = smalls.tile([P, 1], fp32)
            nc.vector.tensor_copy(out=b_abs, in_=b_sb[:, 0, 0:1])

            xT = [acts.tile([P, Q], fp32, tag="actT", name=f"xT{h}") for h in range(2)]
            with tc.tile_pool(name="tpsum", bufs=2, space="PSUM") as tpsum, \
                 tc.tile_pool(name="mlpp", bufs=3, space="PSUM") as mlpp:
                for t in range(QC):
                    for h in range(2):
                        pst = tpsum.tile([P, P], fp32)
                        nc.tensor.transpose(
                            pst, xn[t // 4][:, t % 4, h * P : (h + 1) * P], identity
                        )
                        if (t * 2 + h) % 2 == 0:
                            nc.scalar.copy(
                                out=xT[h][:, t * P : (t + 1) * P], in_=pst
                            )
                        else:
                            nc.vector.tensor_copy(
                                out=xT[h][:, t * P : (t + 1) * P], in_=pst
                            )

                cur = xT
                for layer in range(3):
                    nxt = [
                        acts.tile([P, Q], fp32, tag="actT", name=f"y{layer}T{f2}")
                        for f2 in range(2)
                    ]
                    for fc in range(2):
                        for rc in range(2):
                            ps = mlpp.tile([P, 512], fp32)
                            for kc in range(2):
                                nc.tensor.matmul(
                                    ps,
                                    lhsT=w_sb[:, layer, kc, fc * P : (fc + 1) * P],
                                    rhs=cur[kc][:, rc * 512 : (rc + 1) * 512],
                                    start=(kc == 0),
                                    stop=(kc == 1),
                                )
                            if layer < 2:
                                nc.vector.tensor_scalar(
                                    out=nxt[fc][:, rc * 512 : (rc + 1) * 512],
                                    in0=ps,
                                    scalar1=b_sb[:, layer, fc : fc + 1],
                                    scalar2=0.0,
                                    op0=OP.add,
                                    op1=OP.max,
                                )
                            else:
                                nc.vector.tensor_scalar(
                                    out=nxt[fc][:, rc * 512 : (rc + 1) * 512],
                                    in0=ps,
                                    scalar1=b_sb[:, layer, fc : fc + 1],
                                    scalar2=None,
                                    op0=OP.add,
                                )
                    cur = nxt
            meT = cur

            smask = maps.tile([P, FMAP], fp32, tag="maps")
            out_r = out_d[:, :, :].rearrange("h (t p) k -> h p t k", p=P)
            spsum = ctx.enter_context(
                tc.tile_pool(name="spsum", bufs=4, space="PSUM")
            )

            chain = maps.tile([P, FMAP], fp32, tag="maps")
            attn = chain
            for qc in range(QC):
                ps = spsum.tile([P, Q], fp32)
                for nh in range(2):
                    for kc in range(2):
                        nc.tensor.matmul(
                            ps[:, nh * 512 : (nh + 1) * 512],
                            lhsT=meT[kc][:, qc * P : (qc + 1) * P],
                            rhs=meT[kc][:, nh * 512 : (nh + 1) * 512],
                            start=(kc == 0),
                            stop=(kc == 1),
                        )
                nc.scalar.activation(
                    out=attn[:, qc * Q : (qc + 1) * Q],
                    in_=ps,
                    func=AF.Sigmoid,
                    scale=1.0 / math.sqrt(D),
                )

            rs = smalls.tile([P, QC], fp32)
            neg_inv = smalls.tile([P, QC], fp32)
            e_thresh = float(np.exp(np.float32(THRESHOLD * exp_scale)))
            for qc in range(QC):
                sl = slice(qc * Q, (qc + 1) * Q)
                nc.scalar.activation(
                    out=smask[:, sl],
                    in_=attn[:, sl],
                    func=AF.Sigmoid,
                    scale=-100.0,
                    bias=c50_t,
                )
            for qc in range(QC):
                sl = slice(qc * Q, (qc + 1) * Q)
                nc.scalar.activation(
                    out=chain[:, sl], in_=chain[:, sl], func=AF.Exp,
                    scale=exp_scale,
                )
                nc.vector.scalar_tensor_tensor(
                    out=chain[:, sl],
                    in0=chain[:, sl],
                    scalar=e_thresh,
                    in1=chain[:, sl],
                    op0=OP.is_lt,
                    op1=OP.mult,
                    accum_out=rs[:, qc : qc + 1],
                )
            nc.vector.tensor_scalar(
                out=neg_inv,
                in0=rs,
                scalar1=-1.0,
                scalar2=-BP_EPS,
                op0=OP.mult,
                op1=OP.add,
            )
            nc.vector.reciprocal(out=neg_inv, in_=neg_inv)
            for qc in range(QC):
                sl = slice(qc * Q, (qc + 1) * Q)
                nc.scalar.activation(
                    out=chain[:, sl],
                    in_=chain[:, sl],
                    func=AF.Ln,
                    scale=neg_inv[:, qc : qc + 1],
                    bias=1.0,
                )
                nc.scalar.activation(
                    out=chain[:, sl],
                    in_=chain[:, sl],
                    func=AF.Exp,
                    scale=float(N_SAMPLES),
                )
            mp = chain

            dve_abs = smalls.tile([P, 2], fp32)
            pool_abs = smalls.tile([P, 2], fp32)
            nc.vector.tensor_copy(out=dve_abs[:, 0:1], in_=rand_sb[0][:, 0:1])
            nc.vector.tensor_copy(out=dve_abs[:, 1:2], in_=rand_sb[1][:, 0:1])
            nc.gpsimd.tensor_copy(out=pool_abs[:, 0:1], in_=rand_sb[0][:, 0:1])
            nc.gpsimd.tensor_copy(out=pool_abs[:, 1:2], in_=rand_sb[1][:, 0:1])

            work = [
                maps.tile([P, FMAP], fp32, tag="maps", name=f"work{h}")
                for h in range(2)
            ]
            for h in range(2):
                sub_eng = nc.vector if h == 0 else nc.gpsimd
                for qc in range(QC):
                    sl = slice(qc * Q, (qc + 1) * Q)
                    sub_eng.tensor_tensor(
                        out=work[h][:, sl],
                        in0=mp[:, sl],
                        in1=rand_sb[h][:, sl],
                        op=OP.subtract,
                    )
                    nc.scalar.activation(
                        out=rand_sb[h][:, sl],
                        in_=work[h][:, sl],
                        func=AF.Sigmoid,
                        scale=100.0,
                    )
                    mul_eng = nc.vector if h == 0 else nc.gpsimd
                    mul_eng.tensor_tensor(
                        out=work[h][:, sl],
                        in0=smask[:, sl],
                        in1=rand_sb[h][:, sl],
                        op=OP.mult,
                    )
                    nc.sync.dma_start(
                        out=out_r[h, :, qc, :],
                        in_=work[h][:, sl],
                    )

    return _legalize_waits(nc)


def _get_nc(layer_id: int, fast: bool):
    key = (int(layer_id), bool(fast))
    if key not in _BUILD_CACHE:
        if fast:
            _BUILD_CACHE[key] = _build_fast16()
        else:
            _BUILD_CACHE[key] = _build_full(int(layer_id))
    return _BUILD_CACHE[key]


def _fast_path_ok(tgt_mask, w_all, b_all):
    """Host-side guard for the fast16 variant: verify on the actual inputs
    that every score is > 0.45, i.e. attn > 0.61, so bp == 0, masked_prob
    == 1 exactly, and t = 1-sigmoid(100(attn-.5)) < 2^-13 -- the fp16
    subnormal grid then reproduces the reference's 2^-23 bucket grid. Also
    verify the fp16-matmul pipeline lands in the same buckets as the f32
    one (flip margin check)."""
    x = tgt_mask.astype(np.float16).astype(np.float32)
    mu = x.mean(-1, keepdims=True)
    var = x.var(-1, keepdims=True)
    xn = ((x - mu) / np.sqrt(var + LN_EPS)).astype(np.float32)

    def mlp(xi, dt):
        def r(a):
            return a.astype(dt).astype(np.float32) if dt else a

        h = np.maximum(r(xi) @ r(w_all[0]) + b_all[0], 0.0).astype(np.float32)
        h = np.maximum(r(h) @ r(w_all[1]) + b_all[1], 0.0).astype(np.float32)
        return (r(h) @ r(w_all[2]) + b_all[2]).astype(np.float32)

    me32 = mlp(xn, None)
    me16 = mlp(xn, np.float16)
    for b in range(me32.shape[0]):
        s32 = (me32[b] @ me32[b].T) / np.float32(math.sqrt(D))
        if float(s32.min()) <= 0.45:
            return False
        s16 = (
            me16[b].astype(np.float16).astype(np.float32)
            @ me16[b].astype(np.float16).astype(np.float32).T
        ) / np.float32(math.sqrt(D))
        # bucket comparison: t/2 on the fp16 subnormal grid
        a32 = (1.0 / (1.0 + np.exp(-s32))).astype(np.float32)
        a16 = (1.0 / (1.0 + np.exp(-s16))).astype(np.float32)
        t32 = np.exp((np.float32(50 - LN2F) - 100 * a32).astype(np.float32))
        t16 = np.exp((np.float32(50 - LN2F) - 100 * a16).astype(np.float32))
        q32 = t32.astype(np.float16)
        q16 = t16.astype(np.float16)
        nflip = int((q32 != q16).sum())
        if nflip > 1:
            return False
    return True


def _run(
    tgt_mask,
    ln_w,
    ln_b,
    w1,
    b1,
    w2,
    b2,
    w3,
    b3,
    rand,
    layer_id,
    trace=False,
    force_path=None,
):
    from concourse.bass_utils import run_bass_kernel_spmd

    tgt_mask = np.asarray(tgt_mask, np.float32)
    ln_w = np.asarray(ln_w, np.float32)
    ln_b = np.asarray(ln_b, np.float32)
    w1 = np.asarray(w1, np.float32)
    b1 = np.asarray(b1, np.float32)
    w2 = np.asarray(w2, np.float32)
    b2 = np.asarray(b2, np.float32)
    w3 = np.asarray(w3, np.float32)
    b3 = np.asarray(b3, np.float32)
    rand = np.asarray(rand, np.float32)
    lid = int(np.asarray(layer_id))

    # Fold the layernorm affine params into layer 1: LN(x)*g + c then @w1+b1
    # == LN(x) @ (g[:,None]*w1) + (c@w1 + b1).
    w1f = (ln_w[:, None] * w1).astype(np.float32)
    b1f = (ln_b @ w1 + b1).astype(np.float32)
    w_all = np.ascontiguousarray(np.stack([w1f, w2, w3]), np.float32)
    b_all = np.ascontiguousarray(np.stack([b1f, b2, b3]), np.float32)

    if force_path is None:
        fast = _fast_path_ok(tgt_mask, w_all, b_all)
    else:
        fast = force_path == "fast"
    nc = _get_nc(lid, fast)

    if fast:
        w_dev = np.ascontiguousarray(w_all.astype(np.float16))
        rand_dev = rand.astype(np.float16)
        x_dev = tgt_mask.astype(np.float16)
    else:
        w_dev = w_all
        rand_dev = rand
        x_dev = tgt_mask

    in_maps = []
    for c in range(N_CORES):
        b = c // (N_CORES // B)
        in_maps.append(
            {
                "x": np.ascontiguousarray(x_dev[b]),
                "w": w_dev,
                "b": b_all,
                "rand": np.ascontiguousarray(
                    rand_dev[c * HEADS_PER_CORE : (c + 1) * HEADS_PER_CORE]
                ),
            }
        )

    res = run_bass_kernel_spmd(
        nc, in_maps, core_ids=list(range(N_CORES)), trace=trace
    )
    out = np.concatenate(
        [np.asarray(res.results[c]["out"]) for c in range(N_CORES)], axis=0
    )
    return np.ascontiguousarray(out.astype(np.float32)), res


def kernel(**inputs):
    out, _ = _run(**inputs)
    return out

